# revision 31
# baseline (speedup 1.0000x reference)
"""Multi-head self-attention (2D RoPE) Trainium2 Bass kernel.

Problem: x[4,512,64,64], w_qkv[1536,512], w_proj[512,512], 8 heads, hd=64,
N=4096 positions.  out = proj(attn(rope(q), rope(k)) @ v).

Sharding (8 cores): core c -> batch b=c//2, head-group g=c%2 (heads 4g..4g+3).
Each core computes a partial projection output [512, 4096] over its 256
attention-output channels; host sums the two partials per batch (the
"all-reduce" of the tensor-parallel split) and reshapes.

Per-core design (v2):
 - QKV projection with host-transposed weights; RoPE folded into extra weight
   columns (Jq = J@Wq) so rotation = q*COS + (Jq x)*SIN, three full-width
   vector ops.  q/k land in PER-CHUNK [128, 512] tiles so attention's tile
   deps are chunk-granular and the S/exp pipeline starts ~6us in, not after
   the whole projection.
 - Per chunk of phase 1, k (and V) are produced BEFORE q chunks 1..7 since
   attention consumes all k m-tiles within the first n-chunk.
 - Attention in transposed layout: S^T[m,n] = k_m . q_n, two heads packed in
   the PE via tile_position rows (0,0)/(64,0) (they stream concurrently).
   The two heads run STAGGERED by D=4 units so their chunk boundaries (psum
   accumulator drain + normalize) never collide; each head gets its own
   1-bank psum accumulator pool.
 - exp() runs on the scalar engine out of PSUM (1/8 scale folded, no
   max-subtraction; |S|/8 <= ~10 so fp32 exp is safe).  A few units per
   32-unit chunk are OFFLOADED to the DVE via a one-op fp16-Schraudolph
   exponential (bits = round(1024*(y+15)-60), y = s*log2e/8, ~1.8% rms on
   those columns only), freeing scalar-engine columns that otherwise pace
   the kernel.
 - PV stationary is [ones | zeros | V] so the matmul emits softmax
   denominators at psum partition 0 and values at partitions 64-127.
 - Projection matmul results are DMA'd to HBM straight from PSUM.
 - Whole datapath fp16 (hosts casts), accumulation fp32 in PSUM.
"""

import numpy as np

import concourse.bass as bass
import concourse.mybir as mybir
import concourse.tile as tile
from concourse import bacc
from concourse.bass import ts
from concourse.bass_utils import run_bass_kernel_spmd

F32 = mybir.dt.float32
BF16 = mybir.dt.bfloat16
FP16 = mybir.dt.float16
I16 = mybir.dt.int16
AF = mybir.ActivationFunctionType
ALU = mybir.AluOpType

B, DIM, H, W = 4, 512, 64, 64
HEADS = 8
HD = 64
MAX_FREQ = 10000.0
N_CORES = 8

N, CH, NS = 4096, 512, 512
NMT = N // 128        # 32 m tiles
NCH = N // CH         # 8 phase-1 chunks
NNS = N // NS         # 8 attention n-chunks
KC = DIM // 128       # 4 contract tiles for qkv proj

D_STAG = 4            # head-B stagger (units)
LAG = 6               # exp -> PV pipeline depth (units)

# Schraudolph exp constants: p = u_a + (2^-.5)*u_b with
#   u_a = fp16bits(round(1024*(y + 14) - CA)), y = s*log2e/8
#   u_b = fp16bits(round(1024*(y + 14.5) - CB))
EXP_C1 = 1024.0 * 0.125 * float(np.log2(np.e))
EXP_B1 = 1024.0 * 15.0 - 60.0


def build_nc():
    """Build the per-core Bass program (identical on all 8 cores)."""
    nc = bacc.Bacc("TRN2", target_bir_lowering=False, debug=False,
                   num_devices=N_CORES)

    x_d = nc.dram_tensor("x", [DIM, N], FP16, kind="ExternalInput").ap()
    wqkv_d = nc.dram_tensor("wqkvT", [DIM, 1024], FP16, kind="ExternalInput").ap()
    # misch = [cos0(512) | sin0(512) | pj(128)]; miscr = [cos1..7(3584) |
    # sin1..7(3584) | wv kc-major(1024) | wp ct-major(1024)] -- packed into
    # two tensors so the whole constant set costs two serial DIRECT2D issues
    # (~650ns each) instead of ~12
    miscli_d = nc.dram_tensor("miscli", [128, 1152], FP16,
                              kind="ExternalInput").ap()
    miscr_d = nc.dram_tensor("miscr", [128, 9216], FP16,
                             kind="ExternalInput").ap()
    out_d = nc.dram_tensor("out", [DIM, N], F32, kind="ExternalOutput").ap()

    # units offloaded from scalar-engine exp to DVE+Pool, per 32-unit chunk
    OFF_P0 = {6, 14, 22}
    OFF_P1 = {4, 9, 14, 19, 24}

    with tile.TileContext(nc) as tc:
        with (
            tc.tile_pool(name="singles", bufs=1) as singles,
            tc.tile_pool(name="qkp", bufs=32) as qkp,
            tc.tile_pool(name="xp", bufs=3) as xp,
            tc.tile_pool(name="ropep", bufs=2) as ropep,
            tc.tile_pool(name="rawp", bufs=2) as rawp,
            tc.tile_pool(name="ptp", bufs=10) as ptp,
            tc.tile_pool(name="nsm", bufs=3) as nsm,
            tc.tile_pool(name="ocp", bufs=4) as ocp,
            tc.tile_pool(name="osb", bufs=2) as osb,
        ):
            miscli = singles.tile([128, 1152], FP16, tag="miscli")
            nc.sync.dma_start(out=miscli[:], in_=miscli_d[:, :])
            # wq split per-kc and pair (lo=pair0, hi=pair1): separate tiles
            # so pair-0's first matmul waits only one 128KB transfer, and
            # separate dma_starts parallelize across queues (~22GB/s each)
            wq_lo = [singles.tile([128, 512], FP16, tag=f"wqlo{kc}",
                                  name=f"wqlo{kc}") for kc in range(KC)]
            wq_hi = [singles.tile([128, 512], FP16, tag=f"wqhi{kc}",
                                  name=f"wqhi{kc}") for kc in range(KC)]
            for kc in range(KC):
                nc.sync.dma_start(out=wq_lo[kc][:],
                                  in_=wqkv_d[ts(kc, 128), 0:512])
            miscr = singles.tile([128, 9216], FP16, tag="miscr")
            nc.sync.dma_start(out=miscr[:], in_=miscr_d[:, :])
            for kc in range(KC):
                nc.sync.dma_start(out=wq_hi[kc][:],
                                  in_=wqkv_d[ts(kc, 128), 512:1024])
            pj_sb = miscli[:, 1024:1152]

            def wv_sb(kc):
                return miscr[:, 7168 + 256 * kc:7168 + 256 * (kc + 1)]

            def wp_sb(ct):
                return miscr[:, 8192 + 512 * ct:8192 + 512 * (ct + 1)]

            # stationary layout [ones(64) | V(64)]: PV then emits 64 copies of
            # the softmax denominator at psum partitions 0:64 -- reciprocal on
            # [64, NS] costs the same as [1, NS] (free-dim bound) and needs NO
            # partition broadcast (gpsimd library reloads stall the pipeline)
            v_sb = singles.tile([128, NMT, 4, 128], FP16, tag="v_sb")
            # one output tile PER CHUNK to keep WAR deps chunk-granular
            outs = [singles.tile([128, 2, NS], FP16, tag=f"outc{c}",
                                 name=f"outc{c}")
                    for c in range(NNS)]
            # per-pair per-chunk q/k rope tiles
            q_t = [[None] * NCH for _ in range(2)]
            k_t = [[None] * NCH for _ in range(2)]

            with (
                tc.tile_pool(name="sp", bufs=2, space="PSUM") as sp,
                tc.tile_pool(name="acca", bufs=1, space="PSUM") as acca,
                tc.tile_pool(name="accb", bufs=1, space="PSUM") as accb,
                tc.tile_pool(name="aux", bufs=2, space="PSUM") as aux,
            ):
                def qkv_half(p, ci, which, with_v=False, fold_j=False):
                    """Produce q_t/k_t[p][ci] (both heads of pair p).

                    Loads its own x/cos/sin chunk copies: tile lifetimes stay
                    within this call, so the chunk pools never serialize the
                    k-production front behind deferred q consumption.

                    fold_j=True: J@W folded into extra weight columns (4 more
                    128-contract matmuls, least latency -- used on the
                    startup-critical pair-0 k path).  fold_j=False: raw
                    projection only; J applied by one PJ-permutation matmul
                    from an sbuf copy (5 matmul-equivalents vs 8).
                    """
                    mo0 = 0 if which == "q" else 2
                    x_t = load_x(xp, ci)
                    cos_t, sin_t = load_cs(ci)
                    wq = wq_lo if p == 0 else wq_hi
                    col = mo0 * 128
                    ps = aux.tile([128, CH], F32, tag="aux")
                    for kc in range(KC):
                        nc.tensor.matmul(
                            ps[:],
                            lhsT=wq[kc][:, col:col + 128],
                            rhs=x_t[kc][:],
                            start=(kc == 0), stop=(kc == KC - 1))
                    dst = qkp.tile([128, CH], FP16, tag="qk",
                                   name=f"{which}{p}c{ci}")
                    t1 = ropep.tile([128, CH], FP16, tag="t1")
                    t2 = ropep.tile([128, CH], FP16, tag="t2")
                    if fold_j:
                        ps2 = aux.tile([128, CH], F32, tag="aux")
                        for kc in range(KC):
                            nc.tensor.matmul(
                                ps2[:],
                                lhsT=wq[kc][:, col + 128:col + 256],
                                rhs=x_t[kc][:],
                                start=(kc == 0), stop=(kc == KC - 1))
                        nc.vector.tensor_mul(t1[:], ps[:], cos_t[:])
                        nc.vector.tensor_mul(t2[:], ps2[:], sin_t[:])
                    else:
                        raw16 = rawp.tile([128, CH], FP16, tag="raw")
                        nc.vector.tensor_copy(raw16[:], ps[:])
                        jps = aux.tile([128, CH], F32, tag="aux")
                        nc.tensor.matmul(jps[:], lhsT=pj_sb,
                                         rhs=raw16[:], start=True, stop=True)
                        nc.vector.tensor_mul(t1[:], raw16[:], cos_t[:])
                        nc.vector.tensor_mul(t2[:], jps[:], sin_t[:])
                    nc.vector.tensor_add(dst[:], t1[:], t2[:])
                    if which == "q":
                        q_t[p][ci] = dst
                    else:
                        k_t[p][ci] = dst
                    if with_v:
                        v_chunk(ci, x_t)

                def v_chunk(ci, x_t):
                    """V for all 4 heads, m-tiles 4ci..4ci+3, [m, d] layout."""
                    # per-chunk ones-block init: one big upfront memset would
                    # be an 8.5us Vector op blocking the first rope chunk
                    nc.vector.memset(v_sb[:, 4 * ci:4 * ci + 4, :, 0:64], 1.0)
                    for j in range(CH // 128):
                        mt = 4 * ci + j
                        vp = aux.tile([128, 4, 64], F32, tag="aux")
                        for kc in range(KC):
                            nc.tensor.matmul(
                                vp[:],
                                lhsT=x_t[kc][:, ts(j, 128)],
                                rhs=wv_sb(kc),
                                start=(kc == 0), stop=(kc == KC - 1))
                        nc.vector.tensor_copy(v_sb[:, mt, :, 64:128], vp[:])

                def load_x(pool, ci):
                    # 4 per-kc tiles on 4 queues, issued from the (otherwise
                    # idle) gpsimd sequencer to spare the sync queue
                    x_t = []
                    for kc in range(KC):
                        t = pool.tile([128, CH], FP16, tag=f"x{kc}",
                                      name=f"x{kc}")
                        nc.gpsimd.dma_start(
                            out=t[:], in_=x_d[ts(kc, 128), ts(ci, CH)])
                        x_t.append(t)
                    return x_t

                def load_cs(ci):
                    if ci == 0:
                        return miscli[:, 0:512], miscli[:, 512:1024]
                    o = (ci - 1) * CH
                    return miscr[:, o:o + CH], miscr[:, 3584 + o:3584 + o + CH]

                def exp_unit(s_t, p_t, c0, c1, offload):
                    """exp of s_t[:, c0:c1] -> p_t[:, c0:c1] (fp16)."""
                    if not offload:
                        nc.scalar.activation(p_t[:, c0:c1], s_t[:, c0:c1],
                                             AF.Exp, scale=0.125)
                        return
                    # single-op DVE Schraudolph exp: fp16 bits =
                    # round(1024*(y+15) - 60), y = s*log2e/8, written through
                    # an int16 view of the fp16 p_t tile (~1.8% rms on these
                    # columns, which the softmax-average washes to ~5e-3)
                    nc.vector.tensor_scalar(
                        out=p_t[:, c0:c1].bitcast(I16), in0=s_t[:, c0:c1],
                        scalar1=EXP_C1, scalar2=EXP_B1,
                        op0=ALU.mult, op1=ALU.add)

                def finish(p, head_lo, acc, ns):
                    """Drain+normalize one head's chunk into outs[ns].

                    acc = [denoms(0:64) | values(64:128)].  The full copy
                    frees the psum bank immediately; the values are then
                    partition-SHIFTED to base 0 (1-input DVE ops can shift
                    partitions; 2-input SBUF ops cannot differ in base), so
                    reciprocal and multiply all run base-0 on the DVE with no
                    gpsimd broadcast (gpsimd library reloads stall ~7us).
                    """
                    oc = ocp.tile([128, NS], F32, tag="oc")
                    nc.vector.tensor_copy(oc[:], acc[:])
                    ocv = ocp.tile([64, NS], F32, tag="ocv")
                    nc.vector.tensor_copy(ocv[:], oc[64:128, :])
                    rec = nsm.tile([64, NS], F32, tag="rec")
                    rsc = nsm.tile([64, NS], F32, tag="rsc")
                    nc.vector.reciprocal_approx_accurate(rec[:], oc[0:64, :],
                                                         rsc[:])
                    r0 = 0 if head_lo else 64
                    nc.vector.tensor_mul(
                        outs[ns][r0:r0 + 64, p, :], ocv[:], rec[:])

                def proj_po(c, po):
                    """Output projection of chunk c, partition-block po."""
                    pp = aux.tile([128, NS], F32, tag="aux")
                    for ct in range(2):
                        nc.tensor.matmul(
                            pp[:],
                            lhsT=wp_sb(ct)[:, ts(po, 128)],
                            rhs=outs[c][:, ct, :],
                            start=(ct == 0), stop=(ct == 1))
                    ot = osb.tile([128, NS], F32, tag="ot")
                    nc.vector.tensor_copy(ot[:], pp[:])
                    h = NS // 2
                    nc.gpsimd.dma_start(
                        out=out_d[ts(po, 128), c * NS:c * NS + h],
                        in_=ot[:, 0:h])
                    nc.sync.dma_start(
                        out=out_d[ts(po, 128), c * NS + h:(c + 1) * NS],
                        in_=ot[:, h:])

                # ---- phase 1 head: pair-0 q0 first, then all k+V chunks ----
                qkv_half(0, 0, "q", fold_j=True)
                for ci in range(NCH):
                    qkv_half(0, ci, "k", with_v=True, fold_j=True)

                # ---- attention unit loop per pair ----
                def attn_pair(p, interleave):
                    """interleave: dict iter -> list of thunks to emit."""
                    NU = NNS * NMT
                    offsets = OFF_P0 if p == 0 else OFF_P1
                    pts = {}
                    acc_cur = {"a": None, "b": None}
                    for it in range(NU + D_STAG + LAG):
                        for th in interleave.pop(it, ()):
                            th()
                        a = it if it < NU else None
                        bu = it - D_STAG if 0 <= it - D_STAG < NU else None
                        if a is not None or bu is not None:
                            s_t = sp.tile([128, 2 * NS], F32, tag="s_t")
                            if a is not None:
                                ns, mt = divmod(a, NMT)
                                nc.tensor.matmul(
                                    s_t[:, 0:NS],
                                    lhsT=k_t[p][mt // 4][0:64, ts(mt % 4, 128)],
                                    rhs=q_t[p][ns][0:64, :],
                                    start=True, stop=True,
                                    tile_position=(0, 0))
                            if bu is not None:
                                ns, mt = divmod(bu, NMT)
                                nc.tensor.matmul(
                                    s_t[:, NS:2 * NS],
                                    lhsT=k_t[p][mt // 4][64:128, ts(mt % 4, 128)],
                                    rhs=q_t[p][ns][64:128, :],
                                    start=True, stop=True,
                                    tile_position=(64, 0))
                            p_t = ptp.tile([128, 2 * NS], FP16, tag="p_t")
                            c0 = 0 if a is not None else NS
                            c1 = 2 * NS if bu is not None else NS
                            off = (a is not None and bu is not None
                                   and (a % NMT) in offsets)
                            exp_unit(s_t, p_t, c0, c1, off)
                            pts[it] = p_t
                        w = it - LAG
                        if w < 0:
                            continue
                        p_t = pts.pop(w, None)
                        wa = w if w < NU else None
                        wb = w - D_STAG if 0 <= w - D_STAG < NU else None
                        if wa is not None:
                            ns, mt = divmod(wa, NMT)
                            if mt == 0:
                                acc_cur["a"] = acca.tile([128, NS], F32, tag="oa", name="oa")
                            nc.tensor.matmul(
                                acc_cur["a"][:],
                                lhsT=v_sb[:, mt, 2 * p + 0, :],
                                rhs=p_t[:, 0:NS],
                                start=(mt == 0), stop=(mt == NMT - 1))
                            if mt == NMT - 1:
                                finish(p, True, acc_cur["a"], ns)
                        if wb is not None:
                            ns, mt = divmod(wb, NMT)
                            if mt == 0:
                                acc_cur["b"] = accb.tile([128, NS], F32, tag="ob", name="ob")
                            nc.tensor.matmul(
                                acc_cur["b"][:],
                                lhsT=v_sb[:, mt, 2 * p + 1, :],
                                rhs=p_t[:, NS:2 * NS],
                                start=(mt == 0), stop=(mt == NMT - 1))
                            if mt == NMT - 1:
                                finish(p, False, acc_cur["b"], ns)

                # pair-0 unit loop: interleave pair-0 q chunks 1..7 early,
                # then pair-1's whole phase 1 spread across the window
                il0 = {}
                for j in range(1, NCH):
                    il0.setdefault(2 + 3 * (j - 1), []).append(
                        (lambda jj: lambda: qkv_half(0, jj, "q"))(j))
                for ci in range(NCH):
                    base = 24 + 26 * ci
                    il0.setdefault(base, []).append(
                        (lambda cc: lambda: qkv_half(1, cc, "k"))(ci))
                    il0.setdefault(base + 13, []).append(
                        (lambda cc: lambda: qkv_half(1, cc, "q"))(ci))
                attn_pair(0, il0)

                # pair-1 unit loop: interleave output projection (needs both
                # pairs' normalized chunks; chunk c ready once pair-1's B head
                # finishes it at drain-iter 32c+31+D_STAG)
                il1 = {}
                for c in range(NNS - 1):
                    for j, po in enumerate(range(4)):
                        il1.setdefault(32 * (c + 1) + D_STAG + LAG
                                       + 2 + 5 * j, []).append(
                            (lambda cc, pp: lambda: proj_po(cc, pp))(c, po))
                attn_pair(1, il1)
                for po in range(4):
                    proj_po(NNS - 1, po)

    nc.compile()
    return nc


def rope_tables(h, w, n):
    """cos/sin lookup tables, tiled x4 along partitions -> [128, n]."""
    quarter = HD // 4  # 16
    pos_h, pos_w = np.meshgrid(np.arange(h, dtype=np.float64),
                               np.arange(w, dtype=np.float64), indexing="ij")
    pos = np.stack([pos_h.ravel(), pos_w.ravel()], axis=-1)[:n]
    freqs = 1.0 / (MAX_FREQ ** (np.arange(quarter, dtype=np.float64) / quarter))
    ang = np.concatenate([pos[:, 0:1] * freqs, pos[:, 1:2] * freqs], axis=-1)
    cos = np.cos(ang).T.astype(np.float32)  # [32, n]
    sin = np.sin(ang).T.astype(np.float32)
    return np.tile(cos, (4, 1)), np.tile(sin, (4, 1))


def host_prep(x, w_qkv, w_proj, n=4096, h=H, w=W):
    """Build the 8 per-core input maps."""
    x = np.asarray(x, dtype=np.float32)
    w_qkv = np.asarray(w_qkv, dtype=np.float32)
    w_proj = np.asarray(w_proj, dtype=np.float32)
    dim = x.shape[1]
    cos128, sin128 = rope_tables(h, w, n)

    def jmat(wh):  # wh [64, dim] -> J @ wh
        return np.concatenate([-wh[32:64], wh[0:32]], axis=0)

    # PJ permutation for the on-device J: out[m] = sum_d PJ[d, m] q[d]
    # (Jq[m] = -q[m+32] for m<32, +q[m-32] for 32<=m<64, per 64-head block)
    pj = np.zeros((128, 128), np.float32)
    for h0 in (0, 64):
        for m in range(32):
            pj[h0 + m + 32, h0 + m] = -1.0
            pj[h0 + m, h0 + m + 32] = 1.0

    in_maps = []
    for c in range(N_CORES):
        b, g = c // 2, c % 2
        hs = [4 * g + i for i in range(4)]
        cols = []
        for pair in range(2):
            h0, h1 = hs[2 * pair], hs[2 * pair + 1]
            wq0, wq1 = w_qkv[64 * h0:64 * h0 + 64], w_qkv[64 * h1:64 * h1 + 64]
            wk0 = w_qkv[dim + 64 * h0: dim + 64 * h0 + 64]
            wk1 = w_qkv[dim + 64 * h1: dim + 64 * h1 + 64]
            cols += [wq0, wq1, jmat(wq0), jmat(wq1),
                     wk0, wk1, jmat(wk0), jmat(wk1)]
        wqkvT = np.concatenate(cols, axis=0).T.copy()  # [dim, 1024]

        wvT = np.zeros((dim, 256), np.float32)
        for i, hh in enumerate(hs):
            wvT[:, 64 * i:64 * i + 64] = w_qkv[2 * dim + 64 * hh:
                                               2 * dim + 64 * hh + 64].T
        wprojT = w_proj[:, 256 * g:256 * g + 256].T.copy()  # [256, dim]

        cos16 = cos128[:, :n].astype(np.float16)
        sin16 = sin128[:, :n].astype(np.float16)
        wv_pk = wvT.reshape(4, 128, 256).transpose(1, 0, 2).reshape(128, 1024)
        wp_pk = wprojT.reshape(2, 128, 512).transpose(1, 0, 2).reshape(128, 1024)
        miscli = np.concatenate(
            [cos16[:, 0:512], sin16[:, 0:512], pj.astype(np.float16)], axis=1)
        miscr = np.concatenate(
            [cos16[:, 512:], sin16[:, 512:],
             wv_pk.astype(np.float16), wp_pk.astype(np.float16)], axis=1)
        in_maps.append({
            "x": np.ascontiguousarray(x[b].reshape(dim, n)).astype(np.float16),
            "wqkvT": np.ascontiguousarray(wqkvT).astype(np.float16),
            "miscli": np.ascontiguousarray(miscli),
            "miscr": np.ascontiguousarray(miscr),
        })
    return in_maps


_NC_CACHE = {}


def kernel(x, w_qkv, w_proj, trace=False):
    key = "full"
    if key not in _NC_CACHE:
        _NC_CACHE[key] = build_nc()
    nc = _NC_CACHE[key]
    in_maps = host_prep(x, w_qkv, w_proj)
    res = run_bass_kernel_spmd(nc, in_maps, list(range(N_CORES)), trace=trace)
    outs = [res.results[c]["out"] for c in range(N_CORES)]
    full = np.empty((B, DIM, H, W), np.float32)
    for b in range(B):
        full[b] = (outs[2 * b] + outs[2 * b + 1]).reshape(DIM, H, W)
    kernel.last_results = res
    return full
